# revision 43
# baseline (speedup 1.0000x reference)
"""Trainium2 Bass kernel for C = triu(A @ B), A/B upper-triangular 4096x4096 f32.

kernel(**inputs) takes FULL inputs {"A","B"} and returns the FULL output,
sharding across 8 NeuronCores via run_bass_kernel_spmd (SPMD: one program,
per-core data).

Design (v3, octet band sweeps, bf16):
  C tiled into 128x512 supers (bi=row-block 0..31, jg=col-group 0..7).
  Contraction bk split into left-aligned 8-bands (trailing 4-band for even
  jg). Work units accumulate into PSUM banks over ONE shared B stream:

    O8S  stair octet: 8 rows a..a+7 over their own band, row r engages at
         step r (8 banks).  12 mid (O8SM) + 4 diag (O8SD, width-padded).
    O8M  full octet: 8 rows above a mid band, all engaged (8 banks). 8 units.
    F8D  full quad over a diag-8 band (4 banks). 12 units + hosts 4 F4D
         right-aligned.
    F4L  full quad over a diag-4 band (4 banks). Hosts 8 F4D + 4 S4D
         (stair zero-padded) + 4 ghosts.

  Per-core slots (identical instruction streams on all cores => balance):
    [O8S, F8D, O8M, F4L, O8S, F8D, F4L]  = 48 steps, 40 bank-evictions.
  All 8 PSUM banks rotate manually (pool bufs=1, tags b0..b7).

  Stream: one DMA per step (sync/gpsimd alternating), packed
  [A tracks | B cols] bf16.  Evictions: psum -> bf16 mega-tile (vector +
  scalar engines split) -> one DMA per slot.  Host sums bf16 partials in
  f32.  Single bf16 matmul per (track, step): rel err ~3e-3 (gate 2e-2).
"""

import sys

sys.path.insert(0, "/opt/trn_rl_repo")

import numpy as np

N = 4096
N_CORES = 8
NB = N // 128
NJ = N // 512

MODE = "bf16"

# ---------------------------------------------------------------- schedule
# Lane (slot template) defs: name -> (L, ntracks, stair, diag_widths)
_LANES = {
    "O8S": (8, 8, True, False),
    "O8M": (8, 8, False, False),
    "F8D": (8, 4, False, True),
    "F4L": (4, 4, False, True),
}

# Slot order sorted by DMA demand per PE-cycle (O8M ~230GB/s < O8S ~289 <
# F8D ~375 < F4L ~400 at full clock): low-demand slots run first so the
# stream prefetch builds headroom before the bandwidth-hungry diag slots.
# Stair slots follow 8-bank slots (bank r needed only at step r = perfect
# eviction overlap); quad slots alternate bank halves so each starts on
# banks drained during the previous slot.
_SLOT_TYPES = ["O8M", "O8S", "O8S", "F8D", "F8D", "F4L", "F4L"]
_SLOT_BANKS = [
    [0, 1, 2, 3, 4, 5, 6, 7],   # O8M
    [0, 1, 2, 3, 4, 5, 6, 7],   # O8S#1: stair absorbs O8M's eviction
    [0, 1, 2, 3, 4, 5, 6, 7],   # O8S#2: stair absorbs O8S#1's eviction
    [0, 1, 2, 3],               # F8D#1: O8S#2's first-evicted banks
    [4, 5, 6, 7],               # F8D#2: drained during F8D#1
    [0, 1, 2, 3],               # F4L#1: drained during F8D#2
    [4, 5, 6, 7],               # F4L#2: drained during F4L#1
]
# per-slot eviction emission order (list indices); None = natural
_SLOT_EVICT_ORDER = [None] * 7


def _lane_widths(ttype):
    L, nt, stair, diag = _LANES[ttype]
    ws = []
    for t in range(L):
        rem = L - 1 - t
        ws.append(128 * (rem + 1) if (diag and rem < 3) else 512)
    return ws


def _lane_e(ttype, t):
    L, nt, stair, diag = _LANES[ttype]
    return (t + 1) if stair else nt


def _bands(R):
    out = []
    a = 0
    while a < R:
        blen = 8 if R - a >= 8 else 4
        out.append((a, a + blen - 1, blen))
        a += blen
    return out


def _enumerate_units():
    """Units: dict(type, jg, a, r0).  a = band start bk, r0 = top row."""
    u = {"O8S": [], "O8M": [], "F8D": [], "F4D": [], "S4D": []}
    for jg in range(NJ):
        R = 4 * jg + 4
        for (a, b, blen) in _bands(R):
            diag = (b == R - 1)
            if blen == 8:
                u["O8S"].append(("O8SD" if diag else "O8SM", jg, a, a))
                if diag:
                    for t in range(a // 4):
                        u["F8D"].append(("F8D", jg, a, 4 * t))
                else:
                    for o in range(a // 8):
                        u["O8M"].append(("O8M", jg, a, 8 * o))
            else:
                u["S4D"].append(("S4D", jg, a, a))
                for t in range(a // 4):
                    u["F4D"].append(("F4D", jg, a, 4 * t))
    assert [len(u[k]) for k in ("O8S", "O8M", "F8D", "F4D", "S4D")] == \
        [16, 8, 12, 12, 4]
    return u


def _build_assignment():
    u = _enumerate_units()
    for k in u:
        u[k].sort(key=lambda q: (q[1], q[2], q[3]))
    queues = {
        "O8S": u["O8S"],                          # 16 -> 2 slots
        "O8M": u["O8M"],                          # 8  -> 1 slot
        "F8D": u["F8D"] + u["F4D"][:4],           # 16 -> 2 slots
        "F4L": u["F4D"][4:] + u["S4D"] + [None] * 4,  # 16 -> 2 slots
    }
    pos = {k: 0 for k in queues}
    assign = [[None] * len(_SLOT_TYPES) for _ in range(N_CORES)]
    for s, ttype in enumerate(_SLOT_TYPES):
        grp = queues[ttype][pos[ttype]:pos[ttype] + 8]
        pos[ttype] += 8
        assert len(grp) == 8, (s, ttype)
        for c in range(N_CORES):
            assign[c][s] = grp[c]
    for k in queues:
        assert pos[k] == len(queues[k]), k
    return assign


_ASSIGN = _build_assignment()
_TOTAL_STEPS = sum(_LANES[t][0] for t in _SLOT_TYPES)  # 48
_TOTAL_BANKS = sum(len(b) for b in _SLOT_BANKS)        # 40

_cache = {}


# DMA grouping: steps per load-DMA for each slot (first slot's groups small
# so the PE can start while the bulk streams; few DMAs overall to stay
# within the Tile semaphore pool and avoid sem-epoch reuse stalls).
_SLOT_GROUPS = [(1, 1, 2, 2, 2), (1, 1, 2, 2, 2), (2, 2, 2, 2), (2, 2, 2, 2),
                (2, 2, 2, 2), (2, 2), (2, 2)]


def _layout():
    """steps[i] = (e, w, tile_id, col_ofs); tiles[j] = (word_ofs, gwpp)."""
    steps, tiles = [], []
    ofs = 0
    for s, ttype in enumerate(_SLOT_TYPES):
        L = _LANES[ttype][0]
        ws = _lane_widths(ttype)
        assert sum(_SLOT_GROUPS[s]) == L
        t = 0
        for g in _SLOT_GROUPS[s]:
            gw = 0
            for k in range(g):
                e = _lane_e(ttype, t + k)
                w = ws[t + k]
                steps.append((e, w, len(tiles), gw))
                gw += 128 * e + w
            tiles.append((ofs, gw))
            ofs += 128 * gw
            t += g
    return steps, tiles, ofs

# ------------------------------------------------------------------ device


def _build_nc():
    import concourse.bacc as bacc
    import concourse.mybir as mybir
    import concourse.tile as tile

    f32 = mybir.dt.float32
    bf16 = mybir.dt.bfloat16
    nc = bacc.Bacc()
    steps_layout, tiles_layout, total_words = _layout()
    s_in = nc.declare_dram_parameter("S", [total_words], bf16, isOutput=False)
    cp = nc.declare_dram_parameter("CP", [128, _TOTAL_BANKS * 512], bf16,
                                   isOutput=True)

    with tile.TileContext(nc) as tc:
        with (
            tc.tile_pool(name="st", bufs=21) as s_pool,
            tc.tile_pool(name="co", bufs=3) as c_pool,
            tc.tile_pool(name="ps", bufs=1, space="PSUM") as ps_pool,
        ):
            cursor = 0
            bank_ofs = 0
            cur_tile = [-1, None]
            for s, ttype in enumerate(_SLOT_TYPES):
                L, nt, stair, diag = _LANES[ttype]
                banks = _SLOT_BANKS[s]
                ps = [ps_pool.tile([128, 512], f32, tag=f"b{bk}",
                                   name=f"ps_{s}_{i}")
                      for i, bk in enumerate(banks)]
                for t in range(L):
                    e, w, tid, cofs = steps_layout[cursor]
                    oc = 512 - w
                    if cur_tile[0] != tid:
                        ofs, gwpp = tiles_layout[tid]
                        src = s_in[ofs:ofs + 128 * gwpp] \
                            .rearrange("(p w) -> p w", p=128)
                        st = s_pool.tile([128, gwpp], bf16, tag="s",
                                         name=f"st_{tid}")
                        ldq = nc.sync if tid % 2 == 0 else nc.gpsimd
                        ldq.dma_start(out=st[:], in_=src[:])
                        cur_tile = [tid, st]
                    st = cur_tile[1]
                    for r in range(nt):
                        first = (t == (r if stair else 0))
                        if stair and t < r:
                            continue
                        nc.tensor.matmul(
                            ps[r][:, oc:],
                            lhsT=st[:, cofs + 128 * r:cofs + 128 * (r + 1)],
                            rhs=st[:, cofs + 128 * e:cofs + 128 * e + w],
                            start=first, stop=(t == L - 1),
                        )
                    cursor += 1
                # eviction: psum -> bf16 mega tile -> one DMA (two for the
                # final slot so the tail drains sooner)
                nbk = len(banks)
                last = (s == len(_SLOT_TYPES) - 1)
                nparts = 2 if last else 1
                eorder = _SLOT_EVICT_ORDER[s] or list(range(nbk))
                for p in range(nparts):
                    lo, hi = (nbk * p) // nparts, (nbk * (p + 1)) // nparts
                    c_t = c_pool.tile([128, 512 * (hi - lo)], bf16, tag="c",
                                      name=f"c_{s}_{p}")
                    for k, i in enumerate(x for x in eorder if lo <= x < hi):
                        dst = c_t[:, 512 * (i - lo):512 * (i - lo + 1)]
                        if k % 2 == 0:
                            nc.vector.tensor_copy(dst, ps[i][:])
                        else:
                            nc.scalar.copy(dst, ps[i][:])
                    nc.scalar.dma_start(
                        out=cp[:, (bank_ofs + lo) * 512:(bank_ofs + hi) * 512],
                        in_=c_t[:])
                bank_ofs += nbk
            assert cursor == _TOTAL_STEPS and bank_ofs == _TOTAL_BANKS
    nc.finalize()
    return nc


def get_nc():
    if "nc" not in _cache:
        _cache["nc"] = _build_nc()
    return _cache["nc"]


# ------------------------------------------------------------------- host


def _make_blocks(A, B):
    import ml_dtypes

    bf = ml_dtypes.bfloat16
    A4 = A.reshape(NB, 128, NB, 128).transpose(0, 2, 3, 1).astype(bf)
    B4 = B.reshape(NB, 128, NJ, 512).transpose(0, 2, 1, 3).astype(bf)
    return A4, B4


def _unit_steps(ttype, q, L):
    """For hosted unit q in a lane of L steps, yield per lane-step t:
    (bk or None, engage_list, unit_w) — bk None => pad step (zeros)."""
    qtype, jg, a, r0 = q
    R = 4 * jg + 4
    ulen = 4 if qtype in ("F4D", "S4D") else 8
    base_t = L - ulen  # right-align (only F4D-in-F8D has L>ulen)
    out = []
    for t in range(L):
        if t < base_t:
            out.append((None, [], 0))
            continue
        bk = a + (t - base_t)
        w = 512 - 128 * max(0, bk - 4 * jg)
        out.append((bk, None, w))
    return out


def _pack_core(c, A4, B4):
    import ml_dtypes

    steps_layout, tiles_layout, total_words = _layout()
    S = np.zeros(total_words, dtype=ml_dtypes.bfloat16)
    cursor = 0
    for s, ttype in enumerate(_SLOT_TYPES):
        L, nt, stair, diag = _LANES[ttype]
        q = _ASSIGN[c][s]
        if q is None:
            cursor += L
            continue
        qtype, jg, a, r0 = q
        usteps = _unit_steps(ttype, q, L)
        for t in range(L):
            e, w, tid, cofs = steps_layout[cursor]
            cursor += 1
            bk, _, uw = usteps[t]
            if bk is None:
                continue
            ofs, gwpp = tiles_layout[tid]
            wpp = 128 * e + w
            row = S[ofs:ofs + 128 * gwpp].reshape(128, gwpp)[
                :, cofs:cofs + wpp]
            # B: unit width uw right-aligned within lane width w
            uoc = 512 - uw
            loc = 512 - w
            row[:, 128 * e + (uoc - loc):128 * e + (512 - loc)] = \
                B4[bk, jg][:, uoc:]
            # A tracks: engaged lane-tracks 0..e-1 = rows r0..r0+e-1
            for r in range(min(e, nt)):
                bi = r0 + r
                # unit-level engagement: stair units engage row r at bk>=bi
                if bk >= bi and bk >= a:
                    row[:, 128 * r:128 * (r + 1)] = A4[bi, bk]
    return S


def _host_accumulate(per_core):
    C = np.zeros((N, N), dtype=np.float32)
    for c in range(N_CORES):
        cpk = per_core[c]  # [_TOTAL_BANKS, 128, 512] f32
        bank_ofs = 0
        for s, ttype in enumerate(_SLOT_TYPES):
            nbk = len(_SLOT_BANKS[s])
            q = _ASSIGN[c][s]
            if q is None:
                bank_ofs += nbk
                continue
            qtype, jg, a, r0 = q
            for r in range(nbk):
                bi = r0 + r
                if bi >= NB or 128 * bi > 512 * jg + 511:
                    continue
                C[128 * bi:128 * (bi + 1),
                  512 * jg:512 * (jg + 1)] += cpk[bank_ofs + r]
            bank_ofs += nbk
    return C


def _get_runner():
    if "runner" in _cache:
        return _cache["runner"]
    import jax
    import ml_dtypes
    from jax.sharding import Mesh, PartitionSpec
    from jax.experimental.shard_map import shard_map
    from concourse import bass2jax

    nc = get_nc()
    bass2jax.install_neuronx_cc_hook()
    partition_name = (nc.partition_id_tensor.name
                      if nc.partition_id_tensor else None)
    out_shape = (128, _TOTAL_BANKS * 512)
    out_aval = jax.core.ShapedArray(out_shape, ml_dtypes.bfloat16)
    in_names = ["S", "CP"]
    if partition_name is not None:
        in_names.append(partition_name)

    def _body(s_arr, zeros):
        operands = [s_arr, zeros]
        if partition_name is not None:
            operands.append(bass2jax.partition_id_tensor())
        outs = bass2jax._bass_exec_p.bind(
            *operands, out_avals=(out_aval,), in_names=tuple(in_names),
            out_names=("CP",), lowering_input_output_aliases=(),
            sim_require_finite=True, sim_require_nnan=True, nc=nc)
        return outs[0]

    devices = jax.devices()[:N_CORES]
    mesh = Mesh(np.asarray(devices), ("core",))
    sharded = jax.jit(
        shard_map(_body, mesh=mesh,
                  in_specs=(PartitionSpec("core"),) * 2,
                  out_specs=PartitionSpec("core"), check_rep=False),
        donate_argnums=(1,), keep_unused=True)
    _cache["runner"] = sharded
    return sharded


def kernel(A: np.ndarray, B: np.ndarray) -> np.ndarray:
    import ml_dtypes

    A = np.asarray(A, dtype=np.float32)
    B = np.asarray(B, dtype=np.float32)
    A4, B4 = _make_blocks(A, B)
    s_all = np.concatenate([_pack_core(c, A4, B4) for c in range(N_CORES)])
    zeros = np.zeros((N_CORES * 128, _TOTAL_BANKS * 512), ml_dtypes.bfloat16)
    runner = _get_runner()
    out = np.asarray(runner(s_all, zeros))
    per_core = out.reshape(N_CORES, 128, _TOTAL_BANKS, 512) \
        .transpose(0, 2, 1, 3).astype(np.float32)
    return _host_accumulate(per_core)


def _make_in_maps(A, B):
    A = np.asarray(A, dtype=np.float32)
    B = np.asarray(B, dtype=np.float32)
    A4, B4 = _make_blocks(A, B)
    return [{"S": _pack_core(c, A4, B4)} for c in range(N_CORES)]


# revision 44
# speedup vs baseline: 1.0338x; 1.0338x over previous
"""Trainium2 Bass kernel for C = triu(A @ B), A/B upper-triangular 4096x4096 f32.

kernel(**inputs) takes FULL inputs {"A","B"} and returns the FULL output,
sharding across 8 NeuronCores via run_bass_kernel_spmd (SPMD: one program,
per-core data).

Design (v3, octet band sweeps, bf16):
  C tiled into 128x512 supers (bi=row-block 0..31, jg=col-group 0..7).
  Contraction bk split into left-aligned 8-bands (trailing 4-band for even
  jg). Work units accumulate into PSUM banks over ONE shared B stream:

    O8S  stair octet: 8 rows a..a+7 over their own band, row r engages at
         step r (8 banks).  12 mid (O8SM) + 4 diag (O8SD, width-padded).
    O8M  full octet: 8 rows above a mid band, all engaged (8 banks). 8 units.
    F8D  full quad over a diag-8 band (4 banks). 12 units + hosts 4 F4D
         right-aligned.
    F4L  full quad over a diag-4 band (4 banks). Hosts 8 F4D + 4 S4D
         (stair zero-padded) + 4 ghosts.

  Per-core slots (identical instruction streams on all cores => balance):
    [O8S, F8D, O8M, F4L, O8S, F8D, F4L]  = 48 steps, 40 bank-evictions.
  All 8 PSUM banks rotate manually (pool bufs=1, tags b0..b7).

  Stream: one DMA per step (sync/gpsimd alternating), packed
  [A tracks | B cols] bf16.  Evictions: psum -> bf16 mega-tile (vector +
  scalar engines split) -> one DMA per slot.  Host sums bf16 partials in
  f32.  Single bf16 matmul per (track, step): rel err ~3e-3 (gate 2e-2).
"""

import sys

sys.path.insert(0, "/opt/trn_rl_repo")

import numpy as np

N = 4096
N_CORES = 8
NB = N // 128
NJ = N // 512

MODE = "bf16"

# ---------------------------------------------------------------- schedule
# Lane (slot template) defs: name -> (L, ntracks, stair, diag_widths)
_LANES = {
    "O8S": (8, 8, True, False),
    "O8M": (8, 8, False, False),
    "F8D": (8, 4, False, True),
    "F4L": (4, 4, False, True),
}

# Slot order sorted by DMA demand per PE-cycle (O8M ~230GB/s < O8S ~289 <
# F8D ~375 < F4L ~400 at full clock): low-demand slots run first so the
# stream prefetch builds headroom before the bandwidth-hungry diag slots.
# Stair slots follow 8-bank slots (bank r needed only at step r = perfect
# eviction overlap); quad slots alternate bank halves so each starts on
# banks drained during the previous slot.
_SLOT_TYPES = ["O8M", "O8S", "O8S", "F8D", "F8D", "F4L", "F4L"]
_SLOT_BANKS = [
    [0, 1, 2, 3, 4, 5, 6, 7],   # O8M
    [0, 1, 2, 3, 4, 5, 6, 7],   # O8S#1: stair absorbs O8M's eviction
    [0, 1, 2, 3, 4, 5, 6, 7],   # O8S#2: stair absorbs O8S#1's eviction
    [0, 1, 2, 3],               # F8D#1: O8S#2's first-evicted banks
    [4, 5, 6, 7],               # F8D#2: drained during F8D#1
    [0, 1, 2, 3],               # F4L#1: drained during F8D#2
    [4, 5, 6, 7],               # F4L#2: drained during F4L#1
]
# per-slot eviction emission order (list indices); None = natural
_SLOT_EVICT_ORDER = [None] * 7


def _lane_widths(ttype):
    L, nt, stair, diag = _LANES[ttype]
    ws = []
    for t in range(L):
        rem = L - 1 - t
        ws.append(128 * (rem + 1) if (diag and rem < 3) else 512)
    return ws


def _lane_e(ttype, t):
    L, nt, stair, diag = _LANES[ttype]
    return (t + 1) if stair else nt


def _bands(R):
    out = []
    a = 0
    while a < R:
        blen = 8 if R - a >= 8 else 4
        out.append((a, a + blen - 1, blen))
        a += blen
    return out


def _enumerate_units():
    """Units: dict(type, jg, a, r0).  a = band start bk, r0 = top row."""
    u = {"O8S": [], "O8M": [], "F8D": [], "F4D": [], "S4D": []}
    for jg in range(NJ):
        R = 4 * jg + 4
        for (a, b, blen) in _bands(R):
            diag = (b == R - 1)
            if blen == 8:
                u["O8S"].append(("O8SD" if diag else "O8SM", jg, a, a))
                if diag:
                    for t in range(a // 4):
                        u["F8D"].append(("F8D", jg, a, 4 * t))
                else:
                    for o in range(a // 8):
                        u["O8M"].append(("O8M", jg, a, 8 * o))
            else:
                u["S4D"].append(("S4D", jg, a, a))
                for t in range(a // 4):
                    u["F4D"].append(("F4D", jg, a, 4 * t))
    assert [len(u[k]) for k in ("O8S", "O8M", "F8D", "F4D", "S4D")] == \
        [16, 8, 12, 12, 4]
    return u


def _build_assignment():
    u = _enumerate_units()
    for k in u:
        u[k].sort(key=lambda q: (q[1], q[2], q[3]))
    queues = {
        "O8S": u["O8S"],                          # 16 -> 2 slots
        "O8M": u["O8M"],                          # 8  -> 1 slot
        "F8D": u["F8D"] + u["F4D"][:4],           # 16 -> 2 slots
        "F4L": u["F4D"][4:] + u["S4D"] + [None] * 4,  # 16 -> 2 slots
    }
    pos = {k: 0 for k in queues}
    assign = [[None] * len(_SLOT_TYPES) for _ in range(N_CORES)]
    for s, ttype in enumerate(_SLOT_TYPES):
        grp = queues[ttype][pos[ttype]:pos[ttype] + 8]
        pos[ttype] += 8
        assert len(grp) == 8, (s, ttype)
        for c in range(N_CORES):
            assign[c][s] = grp[c]
    for k in queues:
        assert pos[k] == len(queues[k]), k
    return assign


_ASSIGN = _build_assignment()
_TOTAL_STEPS = sum(_LANES[t][0] for t in _SLOT_TYPES)  # 48
_TOTAL_BANKS = sum(len(b) for b in _SLOT_BANKS)        # 40

_cache = {}


# DMA grouping: steps per load-DMA for each slot (first slot's groups small
# so the PE can start while the bulk streams; few DMAs overall to stay
# within the Tile semaphore pool and avoid sem-epoch reuse stalls).
_SLOT_GROUPS = [(1, 1, 2, 2, 2), (1, 1, 2, 2, 2), (2, 2, 2, 2), (2, 2, 2, 2),
                (2, 2, 2, 2), (2, 2), (2, 2)]


def _layout():
    """steps[i] = (e, w, tile_id, col_ofs); tiles[j] = (word_ofs, gwpp)."""
    steps, tiles = [], []
    ofs = 0
    for s, ttype in enumerate(_SLOT_TYPES):
        L = _LANES[ttype][0]
        ws = _lane_widths(ttype)
        assert sum(_SLOT_GROUPS[s]) == L
        t = 0
        for g in _SLOT_GROUPS[s]:
            gw = 0
            for k in range(g):
                e = _lane_e(ttype, t + k)
                w = ws[t + k]
                steps.append((e, w, len(tiles), gw))
                gw += 128 * e + w
            tiles.append((ofs, gw))
            ofs += 128 * gw
            t += g
    return steps, tiles, ofs

# ------------------------------------------------------------------ device


def _build_nc():
    import concourse.bacc as bacc
    import concourse.mybir as mybir
    import concourse.tile as tile

    f32 = mybir.dt.float32
    bf16 = mybir.dt.bfloat16
    nc = bacc.Bacc()
    steps_layout, tiles_layout, total_words = _layout()
    s_in = nc.declare_dram_parameter("S", [total_words], bf16, isOutput=False)
    cp = nc.declare_dram_parameter("CP", [128, _TOTAL_BANKS * 512], bf16,
                                   isOutput=True)

    with tile.TileContext(nc) as tc:
        with (
            tc.tile_pool(name="st", bufs=21) as s_pool,
            tc.tile_pool(name="co", bufs=3) as c_pool,
            tc.tile_pool(name="ps", bufs=1, space="PSUM") as ps_pool,
        ):
            cursor = 0
            bank_ofs = 0
            cur_tile = [-1, None]
            for s, ttype in enumerate(_SLOT_TYPES):
                L, nt, stair, diag = _LANES[ttype]
                banks = _SLOT_BANKS[s]
                ps = [ps_pool.tile([128, 512], f32, tag=f"b{bk}",
                                   name=f"ps_{s}_{i}")
                      for i, bk in enumerate(banks)]
                for t in range(L):
                    e, w, tid, cofs = steps_layout[cursor]
                    oc = 512 - w
                    if cur_tile[0] != tid:
                        ofs, gwpp = tiles_layout[tid]
                        src = s_in[ofs:ofs + 128 * gwpp] \
                            .rearrange("(p w) -> p w", p=128)
                        st = s_pool.tile([128, gwpp], bf16, tag="s",
                                         name=f"st_{tid}")
                        if tid == 0:
                            # first tile gates the PE start: halve its
                            # latency by splitting across both queues
                            half = gwpp // 2
                            nc.sync.dma_start(out=st[:, :half],
                                              in_=src[:, :half])
                            nc.gpsimd.dma_start(out=st[:, half:],
                                                in_=src[:, half:])
                        else:
                            ldq = nc.sync if tid % 2 == 0 else nc.gpsimd
                            ldq.dma_start(out=st[:], in_=src[:])
                        cur_tile = [tid, st]
                    st = cur_tile[1]
                    for r in range(nt):
                        first = (t == (r if stair else 0))
                        if stair and t < r:
                            continue
                        nc.tensor.matmul(
                            ps[r][:, oc:],
                            lhsT=st[:, cofs + 128 * r:cofs + 128 * (r + 1)],
                            rhs=st[:, cofs + 128 * e:cofs + 128 * e + w],
                            start=first, stop=(t == L - 1),
                        )
                    cursor += 1
                # eviction: psum -> bf16 mega tile -> one DMA (two for the
                # final slot so the tail drains sooner)
                nbk = len(banks)
                last = (s == len(_SLOT_TYPES) - 1)
                nparts = 2 if last else 1
                eorder = _SLOT_EVICT_ORDER[s] or list(range(nbk))
                for p in range(nparts):
                    lo, hi = (nbk * p) // nparts, (nbk * (p + 1)) // nparts
                    c_t = c_pool.tile([128, 512 * (hi - lo)], bf16, tag="c",
                                      name=f"c_{s}_{p}")
                    for k, i in enumerate(x for x in eorder if lo <= x < hi):
                        dst = c_t[:, 512 * (i - lo):512 * (i - lo + 1)]
                        if k % 2 == 0:
                            nc.vector.tensor_copy(dst, ps[i][:])
                        else:
                            nc.scalar.copy(dst, ps[i][:])
                    nc.scalar.dma_start(
                        out=cp[:, (bank_ofs + lo) * 512:(bank_ofs + hi) * 512],
                        in_=c_t[:])
                bank_ofs += nbk
            assert cursor == _TOTAL_STEPS and bank_ofs == _TOTAL_BANKS
    nc.finalize()
    return nc


def get_nc():
    if "nc" not in _cache:
        _cache["nc"] = _build_nc()
    return _cache["nc"]


# ------------------------------------------------------------------- host


def _make_blocks(A, B):
    import ml_dtypes

    bf = ml_dtypes.bfloat16
    A4 = A.reshape(NB, 128, NB, 128).transpose(0, 2, 3, 1).astype(bf)
    B4 = B.reshape(NB, 128, NJ, 512).transpose(0, 2, 1, 3).astype(bf)
    return A4, B4


def _unit_steps(ttype, q, L):
    """For hosted unit q in a lane of L steps, yield per lane-step t:
    (bk or None, engage_list, unit_w) — bk None => pad step (zeros)."""
    qtype, jg, a, r0 = q
    R = 4 * jg + 4
    ulen = 4 if qtype in ("F4D", "S4D") else 8
    base_t = L - ulen  # right-align (only F4D-in-F8D has L>ulen)
    out = []
    for t in range(L):
        if t < base_t:
            out.append((None, [], 0))
            continue
        bk = a + (t - base_t)
        w = 512 - 128 * max(0, bk - 4 * jg)
        out.append((bk, None, w))
    return out


def _pack_core(c, A4, B4):
    import ml_dtypes

    steps_layout, tiles_layout, total_words = _layout()
    S = np.zeros(total_words, dtype=ml_dtypes.bfloat16)
    cursor = 0
    for s, ttype in enumerate(_SLOT_TYPES):
        L, nt, stair, diag = _LANES[ttype]
        q = _ASSIGN[c][s]
        if q is None:
            cursor += L
            continue
        qtype, jg, a, r0 = q
        usteps = _unit_steps(ttype, q, L)
        for t in range(L):
            e, w, tid, cofs = steps_layout[cursor]
            cursor += 1
            bk, _, uw = usteps[t]
            if bk is None:
                continue
            ofs, gwpp = tiles_layout[tid]
            wpp = 128 * e + w
            row = S[ofs:ofs + 128 * gwpp].reshape(128, gwpp)[
                :, cofs:cofs + wpp]
            # B: unit width uw right-aligned within lane width w
            uoc = 512 - uw
            loc = 512 - w
            row[:, 128 * e + (uoc - loc):128 * e + (512 - loc)] = \
                B4[bk, jg][:, uoc:]
            # A tracks: engaged lane-tracks 0..e-1 = rows r0..r0+e-1
            for r in range(min(e, nt)):
                bi = r0 + r
                # unit-level engagement: stair units engage row r at bk>=bi
                if bk >= bi and bk >= a:
                    row[:, 128 * r:128 * (r + 1)] = A4[bi, bk]
    return S


def _host_accumulate(per_core):
    C = np.zeros((N, N), dtype=np.float32)
    for c in range(N_CORES):
        cpk = per_core[c]  # [_TOTAL_BANKS, 128, 512] f32
        bank_ofs = 0
        for s, ttype in enumerate(_SLOT_TYPES):
            nbk = len(_SLOT_BANKS[s])
            q = _ASSIGN[c][s]
            if q is None:
                bank_ofs += nbk
                continue
            qtype, jg, a, r0 = q
            for r in range(nbk):
                bi = r0 + r
                if bi >= NB or 128 * bi > 512 * jg + 511:
                    continue
                C[128 * bi:128 * (bi + 1),
                  512 * jg:512 * (jg + 1)] += cpk[bank_ofs + r]
            bank_ofs += nbk
    return C


def _get_runner():
    if "runner" in _cache:
        return _cache["runner"]
    import jax
    import ml_dtypes
    from jax.sharding import Mesh, PartitionSpec
    from jax.experimental.shard_map import shard_map
    from concourse import bass2jax

    nc = get_nc()
    bass2jax.install_neuronx_cc_hook()
    partition_name = (nc.partition_id_tensor.name
                      if nc.partition_id_tensor else None)
    out_shape = (128, _TOTAL_BANKS * 512)
    out_aval = jax.core.ShapedArray(out_shape, ml_dtypes.bfloat16)
    in_names = ["S", "CP"]
    if partition_name is not None:
        in_names.append(partition_name)

    def _body(s_arr, zeros):
        operands = [s_arr, zeros]
        if partition_name is not None:
            operands.append(bass2jax.partition_id_tensor())
        outs = bass2jax._bass_exec_p.bind(
            *operands, out_avals=(out_aval,), in_names=tuple(in_names),
            out_names=("CP",), lowering_input_output_aliases=(),
            sim_require_finite=True, sim_require_nnan=True, nc=nc)
        return outs[0]

    devices = jax.devices()[:N_CORES]
    mesh = Mesh(np.asarray(devices), ("core",))
    sharded = jax.jit(
        shard_map(_body, mesh=mesh,
                  in_specs=(PartitionSpec("core"),) * 2,
                  out_specs=PartitionSpec("core"), check_rep=False),
        donate_argnums=(1,), keep_unused=True)
    _cache["runner"] = sharded
    return sharded


def kernel(A: np.ndarray, B: np.ndarray) -> np.ndarray:
    import ml_dtypes

    A = np.asarray(A, dtype=np.float32)
    B = np.asarray(B, dtype=np.float32)
    A4, B4 = _make_blocks(A, B)
    s_all = np.concatenate([_pack_core(c, A4, B4) for c in range(N_CORES)])
    zeros = np.zeros((N_CORES * 128, _TOTAL_BANKS * 512), ml_dtypes.bfloat16)
    runner = _get_runner()
    out = np.asarray(runner(s_all, zeros))
    per_core = out.reshape(N_CORES, 128, _TOTAL_BANKS, 512) \
        .transpose(0, 2, 1, 3).astype(np.float32)
    return _host_accumulate(per_core)


def _make_in_maps(A, B):
    A = np.asarray(A, dtype=np.float32)
    B = np.asarray(B, dtype=np.float32)
    A4, B4 = _make_blocks(A, B)
    return [{"S": _pack_core(c, A4, B4)} for c in range(N_CORES)]
